# revision 1
# baseline (speedup 1.0000x reference)
"""AttentionBlock (GroupNorm -> qkv conv1x1 -> 4-head attention -> proj + residual)
on 8 Trainium2 NeuronCores.

Sharding: B*NH = 2*4 = 8 (batch, head) pairs -> one per core.
Each core:
  - GroupNorm(32, 512) over its batch's x (recomputed per core)
  - qkv for its head:  q,k,v = W'[3*128, 512] @ xn   (norm affine + qk scale
    folded into W'/bias on host)
  - scoresT[s,t] = sum_c k[c,s] q[c,t]  (s on partitions -> exp output needs
    no transposes).  No max-subtraction: scores are O(1) for this problem.
  - eT = exp(scoresT) (bf16);  Z[t] via fp16 pairwise add-tree + ones-matmul
  - h_unnorm[c,t] = sum_s v[c,s] eT[s,t]
  - partial[o,t] = w_proj[o, head_slice] @ h_unnorm ; Z shipped to host
Host: out[b] = sum_heads partial/Z + b_proj + x  (gather/unshard).

Pipeline: rounds r=0..4; round r interleaves scores+exp of chunk r with the
attn@v accumulation of chunk r-1 at s-tile granularity so the scalar engine
(exp) never starves while the PE does attn@v / proj.
"""

import math
from contextlib import ExitStack

import ml_dtypes
import numpy as np

import concourse.bacc as bacc
import concourse.bass as bass
import concourse.mybir as mybir
import concourse.tile as tile
from concourse.bass_utils import run_bass_kernel_spmd

C = 512
NH = 4
G = 32
EPS = 1e-5
N = 4096          # H*W
CH = 128          # channels per head
B = 2
NCORES = 8
TCHUNK = 1024     # t-columns processed per chunk
NCHUNK = N // TCHUNK
NST = N // 128    # number of 128-wide s tiles

F16 = mybir.dt.float16
BF16 = mybir.dt.bfloat16
F32 = mybir.dt.float32

TRACE = False
TRACE_CORES = [0]
LAST_RESULT = None


def build_program():
    nc = bacc.Bacc()

    x16 = nc.declare_dram_parameter("x16", [C, N], BF16, isOutput=False)
    wqkvT = nc.declare_dram_parameter("wqkvT", [4, 128, 3 * CH], BF16, isOutput=False)
    bqkv = nc.declare_dram_parameter("bqkv", [128, 3], F32, isOutput=False)
    wprojT = nc.declare_dram_parameter("wprojT", [CH, C], BF16, isOutput=False)
    # group membership matrices: mgrp[p, g] = (p // 16 == g)
    mgrp = nc.declare_dram_parameter("mgrp", [128, 8], BF16, isOutput=False)
    mgrpT = nc.declare_dram_parameter("mgrpT", [8, 128], BF16, isOutput=False)
    partial = nc.declare_dram_parameter("partial", [C, N], F32, isOutput=True)
    zout = nc.declare_dram_parameter("zout", [1, N], F32, isOutput=True)

    with tile.TileContext(nc) as tc, ExitStack() as ctx:
        consts = ctx.enter_context(tc.tile_pool(name="consts", bufs=1))
        gn = ctx.enter_context(tc.tile_pool(name="gn", bufs=1))
        xpool = ctx.enter_context(tc.tile_pool(name="xpool", bufs=4))
        spool = ctx.enter_context(tc.tile_pool(name="spool", bufs=2))
        qkvp = ctx.enter_context(tc.tile_pool(name="qkvp", bufs=1))
        epool = ctx.enter_context(tc.tile_pool(name="epool", bufs=17))
        trpool = ctx.enter_context(tc.tile_pool(name="trpool", bufs=8))
        espool = ctx.enter_context(tc.tile_pool(name="espool", bufs=2))
        zpool = ctx.enter_context(tc.tile_pool(name="zpool", bufs=1))
        hpool = ctx.enter_context(tc.tile_pool(name="hpool", bufs=3))
        opool = ctx.enter_context(tc.tile_pool(name="opool", bufs=3))
        ps_sc = ctx.enter_context(tc.tile_pool(name="ps_sc", bufs=2, space="PSUM"))
        ps_acc = ctx.enter_context(tc.tile_pool(name="ps_acc", bufs=2, space="PSUM"))
        ps_mm2 = ctx.enter_context(tc.tile_pool(name="ps_mm2", bufs=2, space="PSUM"))

        # ---- constants ----
        mgrp_sb = consts.tile([128, 8], BF16, tag="mgrp")
        nc.sync.dma_start(out=mgrp_sb, in_=mgrp[:, :])
        mgrpT_sb = consts.tile([8, 128], BF16, tag="mgrpT")
        nc.sync.dma_start(out=mgrpT_sb, in_=mgrpT[:, :])
        ones_col = consts.tile([128, 1], F16, tag="ones")
        nc.vector.memset(ones_col, 1.0)
        eps_sb = consts.tile([128, 1], F32, tag="eps")
        nc.vector.memset(eps_sb, EPS)

        w_tiles = []
        for kt in range(4):
            wt = consts.tile([128, 3 * CH], BF16, tag=f"wq{kt}", name=f"wt{kt}")
            nc.sync.dma_start(out=wt, in_=wqkvT[kt])
            w_tiles.append(wt)
        bq_sb = consts.tile([128, 3], F32, tag="bq")
        nc.sync.dma_start(out=bq_sb, in_=bqkv[:, :])
        wp_sb = consts.tile([CH, C], BF16, tag="wp")
        nc.sync.dma_start(out=wp_sb, in_=wprojT[:, :])

        # ---- load x tiles + per-channel stats ----
        # tiles 0-2: vector bn_stats; tile 3: scalar Square/Identity accum_out
        stats_all = gn.tile([128, 8], F32, tag="stats_all")
        xt = []
        for i in range(4):
            xti = xpool.tile([128, N], BF16, tag="xt", name=f"xt{i}")
            nc.sync.dma_start(out=xti, in_=x16[128 * i : 128 * (i + 1), :])
            xt.append(xti)
            if i < 3:
                st = spool.tile([128, 8, 6], F32, tag="bst", name=f"bst{i}")
                xv = xti.rearrange("p (s f) -> p s f", f=512)
                for s in range(8):
                    nc.vector.bn_stats(out=st[:, s, :], in_=xv[:, s, :])
                mv = spool.tile([128, 2], F32, tag="mv", name=f"mv{i}")
                nc.vector.bn_aggr(out=mv, in_=st)
                # stats_all[:, i] = channel mean;  stats_all[:, 4+i] = E[x^2]
                nc.vector.tensor_copy(out=stats_all[:, i : i + 1], in_=mv[:, 0:1])
                nc.vector.tensor_mul(
                    out=stats_all[:, 4 + i : 5 + i], in0=mv[:, 0:1], in1=mv[:, 0:1]
                )
                nc.vector.tensor_add(
                    out=stats_all[:, 4 + i : 5 + i],
                    in0=stats_all[:, 4 + i : 5 + i],
                    in1=mv[:, 1:2],
                )
            else:
                sq_scr = qkvp.tile([128, N], BF16, tag="qkv0", name="sq_scr")
                sx2 = spool.tile([128, 1], F32, tag="sx2")
                nc.scalar.activation(
                    out=sq_scr,
                    in_=xti,
                    func=mybir.ActivationFunctionType.Square,
                    accum_out=sx2,
                )
                sx1 = spool.tile([128, 1], F32, tag="sx1")
                nc.scalar.activation(
                    out=xti,
                    in_=xti,
                    func=mybir.ActivationFunctionType.Identity,
                    accum_out=sx1,
                )
                nc.vector.tensor_scalar_mul(
                    out=stats_all[:, 3:4], in0=sx1, scalar1=1.0 / N
                )
                nc.vector.tensor_scalar_mul(
                    out=stats_all[:, 7:8], in0=sx2, scalar1=1.0 / N
                )

        # ---- cross-partition group aggregation via PE ----
        stats16 = gn.tile([128, 8], BF16, tag="stats16")
        nc.vector.tensor_copy(out=stats16, in_=stats_all)
        ps_t = ps_mm2.tile([8, 8], F32, tag="mm2")
        nc.tensor.matmul(ps_t, lhsT=mgrp_sb, rhs=stats16, start=True, stop=True)
        gs = gn.tile([8, 8], F32, tag="gs8")
        nc.scalar.mul(out=gs, in_=ps_t, mul=1.0 / 16.0)
        # gvals cols 0..3 = group mean per x-tile, cols 4..7 = group rstd
        gvals = gn.tile([8, 8], F32, tag="gvals")
        nc.vector.tensor_copy(out=gvals[:, 0:4], in_=gs[:, 0:4])
        varg = gn.tile([8, 4], F32, tag="varg")
        nc.vector.tensor_mul(out=varg, in0=gs[:, 0:4], in1=gs[:, 0:4])  # mu^2
        nc.vector.tensor_sub(out=varg, in0=gs[:, 4:8], in1=varg)  # var
        nc.scalar.activation(
            out=varg,
            in_=varg,
            func=mybir.ActivationFunctionType.Sqrt,
            bias=eps_sb[0:8, :],
        )
        nc.vector.reciprocal(out=gvals[:, 4:8], in_=varg)  # rstd
        gvals16 = gn.tile([8, 8], BF16, tag="gvals16")
        nc.vector.tensor_copy(out=gvals16, in_=gvals)
        ps_t2 = ps_mm2.tile([128, 8], F32, tag="mm2")
        nc.tensor.matmul(ps_t2, lhsT=mgrpT_sb, rhs=gvals16, start=True, stop=True)
        sc_all = gn.tile([128, 8], F32, tag="scall")
        nc.vector.tensor_copy(out=sc_all, in_=ps_t2)

        # ---- apply normalization in place: xn = (x - mu) * rstd ----
        for i in range(4):
            nc.vector.tensor_scalar(
                out=xt[i],
                in0=xt[i],
                scalar1=sc_all[:, i : i + 1],
                scalar2=sc_all[:, 4 + i : 5 + i],
                op0=mybir.AluOpType.subtract,
                op1=mybir.AluOpType.mult,
            )

        # ---- qkv = W' @ xn + b', chunk-major, v first so the (serial) vT
        # DMA-xbar transposes start as early as possible ----
        qkv_sb = [None, None, None]
        for j in range(3):
            qkv_sb[j] = qkvp.tile([128, N], BF16, tag=f"qkv{j}", name=f"qkv{j}")
        q_sb, k_sb, v_sb = qkv_sb
        vT = qkvp.tile([128, NST, 128], BF16, tag="vT")
        for ch in range(8):
            for j in (2, 1, 0):  # v, k, q
                ps = ps_acc.tile([128, 512], F32, tag="acc", name=f"qps{j}_{ch}")
                for kt in range(4):
                    nc.tensor.matmul(
                        ps,
                        lhsT=w_tiles[kt][:, j * 128 : (j + 1) * 128],
                        rhs=xt[kt][:, 512 * ch : 512 * (ch + 1)],
                        start=(kt == 0),
                        stop=(kt == 3),
                    )
                nc.scalar.activation(
                    out=qkv_sb[j][:, 512 * ch : 512 * (ch + 1)],
                    in_=ps,
                    func=mybir.ActivationFunctionType.Identity,
                    bias=bq_sb[:, j : j + 1],
                )
            for stt in range(4 * ch, 4 * ch + 4):
                eng = nc.sync if stt % 2 == 0 else nc.scalar
                eng.dma_start_transpose(
                    vT[:, stt, :], v_sb[:, 128 * stt : 128 * (stt + 1)]
                )

        # ---- pipelined rounds: scores+exp(r) interleaved with attn@v(r-1) ----
        ets_prev = None
        for r in range(NCHUNK + 1):
            t0 = r * TCHUNK
            tp = (r - 1) * TCHUNK

            if r >= 1:
                # Z add-tree for chunk r-1 over the 16 pair tiles, emitted up
                # front (vector runs it while PE+ACT stream the st loop);
                # FD=2048 ops, in-place reduction on 8 temps
                tt = []
                for j in range(8):
                    t_ = trpool.tile(
                        [128, 2, TCHUNK], F16, tag="trv", name=f"t{j}"
                    )
                    nc.vector.tensor_add(
                        out=t_, in0=ets_prev[2 * j], in1=ets_prev[2 * j + 1]
                    )
                    tt.append(t_)
                for span in (2, 4, 8):
                    for j in range(0, 8, span):
                        nc.vector.tensor_add(
                            out=tt[j], in0=tt[j], in1=tt[j + span // 2]
                        )
                ps_h = [
                    ps_acc.tile([128, 512], F32, tag="acc", name=f"ps_h{i}")
                    for i in range(2)
                ]

            ets = []
            for stt in range(NST):
                if r < NCHUNK:
                    ps = ps_sc.tile([128, TCHUNK], F32, tag="sc")
                    kslice = k_sb[:, 128 * stt : 128 * (stt + 1)]
                    for hh in range(2):
                        nc.tensor.matmul(
                            ps[:, 512 * hh : 512 * (hh + 1)],
                            lhsT=kslice,
                            rhs=q_sb[:, t0 + 512 * hh : t0 + 512 * (hh + 1)],
                            start=True,
                            stop=True,
                        )
                    if stt % 2 == 0:
                        et = epool.tile([128, 2, TCHUNK], BF16, tag="et")
                        ets.append(et)
                    nc.scalar.activation(
                        out=ets[stt // 2][:, stt % 2, :],
                        in_=ps,
                        func=mybir.ActivationFunctionType.Exp,
                    )
                if r >= 1:
                    ep = ets_prev[stt // 2]
                    for hh in range(2):
                        nc.tensor.matmul(
                            ps_h[hh],
                            lhsT=vT[:, stt, :],
                            rhs=ep[:, stt % 2, 512 * hh : 512 * (hh + 1)],
                            start=(stt == 0),
                            stop=(stt == NST - 1),
                        )

            if r >= 1:
                # finish Z tree, Z matmul, ship Z
                esum = espool.tile([128, TCHUNK], F16, tag="esum")
                nc.vector.tensor_add(
                    out=esum, in0=tt[0][:, 0, :], in1=tt[0][:, 1, :]
                )
                zrow = zpool.tile([1, TCHUNK], F32, tag="zrow")
                for hh in range(2):
                    ps_z = ps_mm2.tile([1, 512], F32, tag="mm2", name=f"ps_z{hh}")
                    nc.tensor.matmul(
                        ps_z,
                        lhsT=ones_col,
                        rhs=esum[:, 512 * hh : 512 * (hh + 1)],
                        start=True,
                        stop=True,
                    )
                    nc.vector.tensor_copy(
                        out=zrow[:, 512 * hh : 512 * (hh + 1)], in_=ps_z
                    )
                nc.sync.dma_start(out=zout[:, tp : tp + TCHUNK], in_=zrow)

                # h_unnorm, proj, store
                for hh in range(2):
                    h_sb = hpool.tile([128, 512], BF16, tag="h")
                    nc.vector.tensor_copy(out=h_sb, in_=ps_h[hh])
                    for ot in range(4):
                        ps_p = ps_mm2.tile([128, 512], F32, tag="mm2")
                        nc.tensor.matmul(
                            ps_p,
                            lhsT=wp_sb[:, 128 * ot : 128 * (ot + 1)],
                            rhs=h_sb,
                            start=True,
                            stop=True,
                        )
                        ob = opool.tile([128, 512], F32, tag="osb")
                        nc.vector.tensor_copy(out=ob, in_=ps_p)
                        nc.sync.dma_start(
                            out=partial[
                                128 * ot : 128 * (ot + 1),
                                tp + 512 * hh : tp + 512 * (hh + 1),
                            ],
                            in_=ob,
                        )
            ets_prev = ets if r < NCHUNK else None

    if not nc.is_finalized():
        nc.finalize()
    return nc


_NC_CACHE = None


def _get_nc():
    global _NC_CACHE
    if _NC_CACHE is None:
        _NC_CACHE = build_program()
    return _NC_CACHE


def kernel(x, norm_w, norm_b, w_qkv, w_proj, b_proj):
    global LAST_RESULT
    x = np.asarray(x, dtype=np.float32)
    norm_w = np.asarray(norm_w, dtype=np.float32)
    norm_b = np.asarray(norm_b, dtype=np.float32)
    w_qkv = np.asarray(w_qkv, dtype=np.float32)
    w_proj = np.asarray(w_proj, dtype=np.float32)
    b_proj = np.asarray(b_proj, dtype=np.float32)

    s1 = 1.0 / math.sqrt(math.sqrt(CH))
    bf16 = ml_dtypes.bfloat16
    mgrp = (np.arange(128)[:, None] // 16 == np.arange(8)[None, :]).astype(bf16)
    in_maps = []
    for core in range(NCORES):
        b, h = divmod(core, NH)
        # reference layout: head h of batch b uses w_qkv rows
        # [384h:384h+128] (q), [384h+128:384h+256] (k), [384h+256:384h+384] (v)
        rows = w_qkv[384 * h : 384 * (h + 1)]  # (384, 512)
        wfold = rows * norm_w[None, :]  # fold GroupNorm gamma
        bias = rows @ norm_b  # fold GroupNorm beta
        scale_vec = np.concatenate(
            [np.full(128, s1), np.full(128, s1), np.ones(128)]
        ).astype(np.float32)
        wfold = wfold * scale_vec[:, None]
        bias = bias * scale_vec
        wqkvT = np.ascontiguousarray(wfold.T.reshape(4, 128, 384).astype(bf16))
        bqkv = np.ascontiguousarray(bias.reshape(3, 128).T.astype(np.float32))
        wprojT = np.ascontiguousarray(
            w_proj[:, 128 * h : 128 * (h + 1)].T.astype(bf16)
        )
        x16 = np.ascontiguousarray(x[b].reshape(C, N).astype(bf16))
        in_maps.append(
            {
                "x16": x16,
                "wqkvT": wqkvT,
                "bqkv": bqkv,
                "wprojT": wprojT,
                "mgrp": mgrp,
                "mgrpT": np.ascontiguousarray(mgrp.T),
            }
        )

    nc = _get_nc()
    res = run_bass_kernel_spmd(
        nc,
        in_maps,
        list(range(NCORES)),
        trace=TRACE,
        trace_cores=TRACE_CORES if TRACE else None,
    )
    LAST_RESULT = res

    out = np.empty((B, C, N), dtype=np.float32)
    for b in range(B):
        acc = x[b].reshape(C, N) + b_proj[:, None]
        for h in range(NH):
            r = res.results[4 * b + h]
            acc = acc + r["partial"] / r["zout"]
        out[b] = acc
    return out.reshape(B, C, 64, 64)



# revision 6
# speedup vs baseline: 1.0055x; 1.0055x over previous
"""AttentionBlock (GroupNorm -> qkv conv1x1 -> 4-head attention -> proj + residual)
on 8 Trainium2 NeuronCores.

Sharding: B*NH = 2*4 = 8 (batch, head) pairs -> one per core.

Per core (head h of batch b):
  - x shipped as fp8e4 [512, 4096] in kt-pair layout (DMA halved vs bf16)
  - GroupNorm folded into runtime weight scaling: W'' = W'(gamma,s1-folded)
    * rstd[channel]; bias = -W''@mu (+ beta fold) applied during the qk
    PSUM->SBUF copies on DVE.  No normalization pass over x.
  - qkv GEMM in fp8 DoubleRow (contraction 256/pass).  q rows folded x4,
    k rows /4 so fp8 weights stay in the normal range (cancels in q.k).
  - scoresT[s,t] = k_tile.T @ q (bf16, s on partitions)
  - exp on ACT only: fp8e4 out with +2.8 offset (cancels in partial/Z),
    written into [128, 2, TCHUNK] pair tiles
  - attn@v: fp8 DoubleRow over s-tile pairs (2x MACs/cycle)
  - Z[t]: pairs 0..Z_PE-1 summed by DoubleRow ones-matmuls on PE; pairs
    Z_PE..15 by a DVE fp16 add tree -> esum; esum joined into the same
    PSUM group by an fp16 ones-matmul.
  - proj on PE (bf16); partial shipped bf16; v-bias compensated on host
    (attention rows sum to 1, so it shifts h/Z by exactly bias_v).
Host: out[b] = x + b_proj + sum_h (partial_h/Z_h + Wp_h @ bias_v_h).
"""

import math
from contextlib import ExitStack

import ml_dtypes
import numpy as np

import concourse.bacc as bacc
import concourse.bass as bass
import concourse.mybir as mybir
import concourse.tile as tile
from concourse.bass_utils import run_bass_kernel_spmd

C = 512
NH = 4
G = 32
EPS = 1e-5
N = 4096
CH = 128
B = 2
NCORES = 8
TCHUNK = 1024
NCHUNK = N // TCHUNK     # 4
NST = N // 128           # 32
NPAIR = NST // 2         # 16
Z_PE = 10                # s-tile pairs of Z summed on PE (rest on DVE tree)
EXPB = 2.8               # exp offset: e^(score+2.8) stays within fp8e4 range
ALPHA = 4.0              # q-row fold (k rows get 1/ALPHA); cancels in q.k

F32 = mybir.dt.float32
BF16 = mybir.dt.bfloat16
F16 = mybir.dt.float16
F8 = mybir.dt.float8e4
DR = mybir.MatmulPerfMode.DoubleRow
AF = mybir.ActivationFunctionType

TRACE = False
TRACE_CORES = [0]
LAST_RESULT = None


def build_program():
    nc = bacc.Bacc()

    # x2[pr, p, i, n] = x[128*(2pr+i)+p, n]  (kt-pair layout for DoubleRow)
    x2d = nc.declare_dram_parameter("x2", [2, 128, 2, N], F8, isOutput=False)
    # wq2[pr, p, i, o] = W'[o, 128*(2pr+i)+p] (gamma, s1, alpha folded)
    wq2d = nc.declare_dram_parameter("wq2", [2, 128, 2, 384], BF16, isOutput=False)
    bqkv = nc.declare_dram_parameter("bqkv", [128, 2], F32, isOutput=False)
    wprojT = nc.declare_dram_parameter("wprojT", [CH, C], BF16, isOutput=False)
    mgrp = nc.declare_dram_parameter("mgrp", [128, 8], BF16, isOutput=False)
    mgrpT = nc.declare_dram_parameter("mgrpT", [8, 128], BF16, isOutput=False)
    partial = nc.declare_dram_parameter("partial", [C, N], BF16, isOutput=True)
    zout = nc.declare_dram_parameter("zout", [1, N], F32, isOutput=True)

    with tile.TileContext(nc) as tc, ExitStack() as ctx:
        consts = ctx.enter_context(tc.tile_pool(name="consts", bufs=1))
        gn = ctx.enter_context(tc.tile_pool(name="gn", bufs=1))
        xpool = ctx.enter_context(tc.tile_pool(name="xpool", bufs=2))
        spool = ctx.enter_context(tc.tile_pool(name="spool", bufs=2))
        qkp = ctx.enter_context(tc.tile_pool(name="qkp", bufs=1))
        epool = ctx.enter_context(tc.tile_pool(name="epool", bufs=32))
        trpool = ctx.enter_context(tc.tile_pool(name="trpool", bufs=14))
        hpool = ctx.enter_context(tc.tile_pool(name="hpool", bufs=2))
        opool = ctx.enter_context(tc.tile_pool(name="opool", bufs=3))
        zsp = ctx.enter_context(tc.tile_pool(name="zsp", bufs=1))
        ps_sc = ctx.enter_context(tc.tile_pool(name="ps_sc", bufs=2, space="PSUM"))
        ps_acc = ctx.enter_context(tc.tile_pool(name="ps_acc", bufs=2, space="PSUM"))
        ps_mm2 = ctx.enter_context(tc.tile_pool(name="ps_mm2", bufs=2, space="PSUM"))

        # ---- constants / weights (gpsimd queue so sync handles x) ----
        mgrp_sb = consts.tile([128, 8], BF16, tag="mgrp")
        nc.gpsimd.dma_start(out=mgrp_sb, in_=mgrp[:, :])
        mgrpT_sb = consts.tile([8, 128], BF16, tag="mgrpT")
        nc.gpsimd.dma_start(out=mgrpT_sb, in_=mgrpT[:, :])
        wbf = []
        for pr in range(2):
            wt = consts.tile([128, 2, 384], BF16, tag=f"wb{pr}")
            nc.gpsimd.dma_start(out=wt, in_=wq2d[pr])
            wbf.append(wt)
        bq_sb = consts.tile([128, 2], F32, tag="bq")
        nc.gpsimd.dma_start(out=bq_sb, in_=bqkv[:, :])
        wp_sb = consts.tile([CH, C], BF16, tag="wp")
        nc.gpsimd.dma_start(out=wp_sb, in_=wprojT[:, :])
        ones2 = consts.tile([128, 2, 16], F8, tag="ones2")
        nc.vector.memset(ones2, 1.0)
        ones1 = consts.tile([128, 1], F16, tag="ones1")
        nc.vector.memset(ones1, 1.0)
        bexp = consts.tile([128, 1], F32, tag="bexp")
        nc.vector.memset(bexp, EXPB)
        beps = consts.tile([8, 1], F32, tag="beps")
        nc.vector.memset(beps, EPS)

        # ---- x loads + per-channel stats (tiles 0,1 on DVE; 2,3 on ACT) ----
        xt = []
        for pr in range(2):
            xti = xpool.tile([128, 2, N], F8, tag="xt", name=f"x{pr}")
            nc.sync.dma_start(out=xti[:, 0, :], in_=x2d[pr, :, 0, :])
            nc.sync.dma_start(out=xti[:, 1, :], in_=x2d[pr, :, 1, :])
            xt.append(xti)

        stats_all = gn.tile([128, 8], F32, tag="stats_all")
        sq_scr = gn.tile([128, N], F8, tag="sq_scr")
        for kt in range(4):
            pr, i = divmod(kt, 2)
            xsl = xt[pr][:, i, :]
            if kt < 2:
                st = spool.tile([128, 8, 6], F32, tag="bst", name=f"bst{kt}")
                xv = xsl.rearrange("p (s f) -> p s f", f=512)
                for s in range(8):
                    nc.vector.bn_stats(out=st[:, s, :], in_=xv[:, s, :])
                mv = spool.tile([128, 2], F32, tag="mv", name=f"mv{kt}")
                nc.vector.bn_aggr(out=mv, in_=st)
                nc.vector.tensor_copy(out=stats_all[:, kt : kt + 1], in_=mv[:, 0:1])
                nc.vector.tensor_mul(
                    out=stats_all[:, 4 + kt : 5 + kt], in0=mv[:, 0:1], in1=mv[:, 0:1]
                )
                nc.vector.tensor_add(
                    out=stats_all[:, 4 + kt : 5 + kt],
                    in0=stats_all[:, 4 + kt : 5 + kt],
                    in1=mv[:, 1:2],
                )
            else:
                sx2 = spool.tile([128, 1], F32, tag="sx2", name=f"sx2{kt}")
                nc.scalar.activation(
                    out=sq_scr, in_=xsl, func=AF.Square, accum_out=sx2
                )
                sx1 = spool.tile([128, 1], F32, tag="sx1", name=f"sx1{kt}")
                nc.scalar.activation(
                    out=sq_scr, in_=xsl, func=AF.Identity, accum_out=sx1
                )
                nc.vector.tensor_scalar_mul(
                    out=stats_all[:, kt : kt + 1], in0=sx1, scalar1=1.0 / N
                )
                nc.vector.tensor_scalar_mul(
                    out=stats_all[:, 4 + kt : 5 + kt], in0=sx2, scalar1=1.0 / N
                )

        # ---- group aggregation: mean_g, rstd_g -> per-channel sc_all ----
        stats16 = gn.tile([128, 8], BF16, tag="stats16")
        nc.vector.tensor_copy(out=stats16, in_=stats_all)
        ps_t = ps_mm2.tile([8, 8], F32, tag="mm2", name="gnps")
        nc.tensor.matmul(ps_t, lhsT=mgrp_sb, rhs=stats16, start=True, stop=True)
        gs = gn.tile([8, 8], F32, tag="gs8")
        nc.vector.tensor_scalar_mul(out=gs, in0=ps_t, scalar1=1.0 / 16.0)
        gvals = gn.tile([8, 8], F32, tag="gvals")
        nc.vector.tensor_copy(out=gvals[:, 0:4], in_=gs[:, 0:4])
        varg = gn.tile([8, 4], F32, tag="varg")
        nc.vector.tensor_mul(out=varg, in0=gs[:, 0:4], in1=gs[:, 0:4])
        nc.vector.tensor_sub(out=varg, in0=gs[:, 4:8], in1=varg)
        # rstd = exp(-0.5*ln(var+eps)) : stays in the exp/ln ACT table set
        nc.scalar.activation(out=varg, in_=varg, func=AF.Ln, bias=beps)
        nc.scalar.activation(
            out=gvals[:, 4:8], in_=varg, func=AF.Exp, scale=-0.5
        )
        gvals16 = gn.tile([8, 8], BF16, tag="gvals16")
        nc.vector.tensor_copy(out=gvals16, in_=gvals)
        ps_t2 = ps_mm2.tile([128, 8], F32, tag="mm2", name="gnps2")
        nc.tensor.matmul(ps_t2, lhsT=mgrpT_sb, rhs=gvals16, start=True, stop=True)
        sc_all = gn.tile([128, 8], F32, tag="scall")
        nc.vector.tensor_copy(out=sc_all, in_=ps_t2)

        # ---- W'' = W' * rstd (fp8), msc = mean*rstd, qk bias GEMV ----
        w8 = []
        for pr in range(2):
            w8t = qkp.tile([128, 2, 384], F8, tag=f"w8{pr}")
            for i in range(2):
                kt = 2 * pr + i
                nc.vector.tensor_scalar_mul(
                    out=w8t[:, i, :],
                    in0=wbf[pr][:, i, :],
                    scalar1=sc_all[:, 4 + kt : 5 + kt],
                )
            w8.append(w8t)
        msc16 = gn.tile([128, 4], BF16, tag="msc16")
        nc.vector.tensor_mul(out=msc16, in0=sc_all[:, 0:4], in1=sc_all[:, 4:8])
        bias_qk = []
        for j in range(2):
            ps_b = ps_mm2.tile([128, 1], F32, tag="mm2", name=f"gemv{j}")
            for kt in range(4):
                pr, i = divmod(kt, 2)
                nc.tensor.matmul(
                    ps_b,
                    lhsT=wbf[pr][:, i, 128 * j : 128 * (j + 1)],
                    rhs=msc16[:, kt : kt + 1],
                    start=(kt == 0),
                    stop=(kt == 3),
                )
            bj = gn.tile([128, 1], F32, tag=f"bias{j}")
            nc.vector.tensor_sub(out=bj, in0=bq_sb[:, j : j + 1], in1=ps_b)
            bias_qk.append(bj)

        # ---- qkv chunk GEMM helper (DoubleRow) ----
        q_sb = qkp.tile([128, N], BF16, tag="q_sb")
        k_sb = qkp.tile([128, N], BF16, tag="k_sb")
        v_sb = qkp.tile([128, N], BF16, tag="v_sb")
        vT = qkp.tile([128, NST, 128], BF16, tag="vT")
        vT2 = qkp.tile([128, NST, 128], F8, tag="vT2")

        def qkv_chunk(j, ch):
            ps = ps_mm2.tile([128, 512], F32, tag="mm2", name=f"qkv{j}_{ch}")
            for pr in range(2):
                nc.tensor.matmul(
                    ps,
                    lhsT=w8[pr][:, :, 128 * j : 128 * (j + 1)],
                    rhs=xt[pr][:, :, 512 * ch : 512 * (ch + 1)],
                    start=(pr == 0),
                    stop=(pr == 1),
                    perf_mode=DR,
                )
            dst = (q_sb, k_sb, v_sb)[j]
            if j < 2:
                nc.vector.tensor_scalar(
                    out=dst[:, 512 * ch : 512 * (ch + 1)],
                    in0=ps,
                    scalar1=bias_qk[j],
                    scalar2=None,
                    op0=mybir.AluOpType.add,
                )
            else:
                nc.vector.tensor_copy(
                    out=dst[:, 512 * ch : 512 * (ch + 1)], in_=ps
                )
                # transpose the 4 s-tiles of this chunk via DMA xbar (sync q)
                for stt in range(4 * ch, 4 * ch + 4):
                    nc.sync.dma_start_transpose(
                        vT[:, stt, :], v_sb[:, 128 * stt : 128 * (stt + 1)]
                    )

        # ---- round machinery ----
        et_all = []          # per round: list of NPAIR pair tiles
        esums = [None] * NCHUNK
        ph_all = [None] * NCHUNK

        def emit_scores(r, stt):
            ps = ps_sc.tile([128, TCHUNK], F32, tag="sc")
            ksl = k_sb[:, 128 * stt : 128 * (stt + 1)]
            t0 = r * TCHUNK
            for hh in range(2):
                nc.tensor.matmul(
                    ps[:, 512 * hh : 512 * (hh + 1)],
                    lhsT=ksl,
                    rhs=q_sb[:, t0 + 512 * hh : t0 + 512 * (hh + 1)],
                    start=True,
                    stop=True,
                )
            if stt % 2 == 0:
                et = epool.tile([128, 2, TCHUNK], F8, tag="et")
                et_all[r].append(et)
            nc.scalar.activation(
                out=et_all[r][stt // 2][:, stt % 2, :],
                in_=ps,
                func=AF.Exp,
                bias=bexp,
            )

        def emit_attnv_pair(r, p):
            ph = ph_all[r]
            for hh in range(2):
                nc.tensor.matmul(
                    ph[hh],
                    lhsT=vT2[:, 2 * p : 2 * p + 2, :],
                    rhs=et_all[r][p][:, :, 512 * hh : 512 * (hh + 1)],
                    start=(p == 0),
                    stop=(p == NPAIR - 1),
                    perf_mode=DR,
                )

        def emit_z(r):
            # both t-halves; PE pairs 0..Z_PE-1 + fp16 esum join
            for hh in range(2):
                zps = ps_mm2.tile([1, 512], F32, tag="mm2", name=f"z{r}_{hh}")
                for p in range(Z_PE):
                    nc.tensor.matmul(
                        zps,
                        lhsT=ones2[:, :, 0:1],
                        rhs=et_all[r][p][:, :, 512 * hh : 512 * (hh + 1)],
                        start=(p == 0),
                        stop=False,
                        perf_mode=DR,
                    )
                nc.tensor.matmul(
                    zps,
                    lhsT=ones1,
                    rhs=esums[r][:, 512 * hh : 512 * (hh + 1)],
                    start=False,
                    stop=True,
                )
                nc.vector.tensor_copy(
                    out=z_sb[:, r * TCHUNK + 512 * hh : r * TCHUNK + 512 * (hh + 1)],
                    in_=zps,
                )

        def emit_tree(r):
            # DVE fp16 tree over pairs Z_PE..15 of round r -> esums[r]
            tt = []
            for p in range(Z_PE, NPAIR):
                t_ = trpool.tile([128, TCHUNK], F16, tag="tr", name=f"t{r}_{p}")
                nc.vector.tensor_add(
                    out=t_, in0=et_all[r][p][:, 0, :], in1=et_all[r][p][:, 1, :]
                )
                tt.append(t_)
            while len(tt) > 1:
                nxt = []
                for a in range(0, len(tt) - 1, 2):
                    nc.vector.tensor_add(out=tt[a], in0=tt[a], in1=tt[a + 1])
                    nxt.append(tt[a])
                if len(tt) % 2 == 1:
                    nxt.append(tt[-1])
                tt = nxt
            esums[r] = tt[0]

        def emit_hcopy_proj(r):
            hsb = hpool.tile([128, TCHUNK], BF16, tag="h")
            for hh in range(2):
                nc.vector.tensor_copy(
                    out=hsb[:, 512 * hh : 512 * (hh + 1)], in_=ph_all[r][hh]
                )
            for ot in range(4):
                for hh in range(2):
                    psp = ps_mm2.tile([128, 512], F32, tag="mm2",
                                      name=f"pj{r}_{ot}_{hh}")
                    nc.tensor.matmul(
                        psp,
                        lhsT=wp_sb[:, 128 * ot : 128 * (ot + 1)],
                        rhs=hsb[:, 512 * hh : 512 * (hh + 1)],
                        start=True,
                        stop=True,
                    )
                    ob = opool.tile([128, 512], BF16, tag="ob")
                    nc.vector.tensor_copy(out=ob, in_=psp)
                    nc.sync.dma_start(
                        out=partial[
                            128 * ot : 128 * (ot + 1),
                            r * TCHUNK + 512 * hh : r * TCHUNK + 512 * (hh + 1),
                        ],
                        in_=ob,
                    )

        z_sb = zsp.tile([1, N], F32, tag="z_sb")

        # ================= round 0 =================
        et_all.append([])
        # k chunk 0, q chunks 0,1 first so scores can start
        qkv_chunk(1, 0)
        qkv_chunk(0, 0)
        qkv_chunk(0, 1)
        emit_scores(0, 0)
        emit_scores(0, 1)
        # stream remaining k chunks just ahead of their score tiles;
        # weave in v chunks (+ transposes) and q chunks 2,3
        fill0 = [("k", ch) for ch in range(1, 8)]
        extra0 = [("v", ch) for ch in range(8)] + [("q", 2), ("q", 3)]
        ei = 0
        for stt in range(2, NST):
            ch = stt // 4 + 1
            if fill0 and stt % 4 == 2 and ch < 8:
                qkv_chunk(1, ch)
                fill0.pop(0)
            if stt % 2 == 1 and ei < len(extra0):
                kind, ch2 = extra0[ei]
                qkv_chunk(2 if kind == "v" else 0, ch2)
                ei += 1
            emit_scores(0, stt)
        while ei < len(extra0):
            kind, ch2 = extra0[ei]
            qkv_chunk(2 if kind == "v" else 0, ch2)
            ei += 1
        # pack vT (bf16) -> vT2 (fp8) for DoubleRow
        for g in range(8):
            nc.vector.tensor_copy(
                out=vT2[:, 4 * g : 4 * (g + 1), :], in_=vT[:, 4 * g : 4 * (g + 1), :]
            )
        emit_tree(0)

        # ================= rounds 1..3 =================
        for r in range(1, NCHUNK):
            et_all.append([])
            ph_all[r - 1] = [
                ps_acc.tile([128, 512], F32, tag="acc", name=f"h{r-1}_{hh}")
                for hh in range(2)
            ]
            if r >= 2:
                emit_hcopy_proj(r - 2)
            # q chunks for rounds 2,3 streamed early in round 1
            pend_q = [("q", 4), ("q", 5), ("q", 6), ("q", 7)] if r == 1 else []
            ap = 0  # attnv pairs emitted
            for stt in range(NST):
                emit_scores(r, stt)
                if pend_q and stt % 2 == 1:
                    qkv_chunk(0, pend_q.pop(0)[1])
                if stt == 10:
                    emit_z(r - 1)
                want = ((stt + 1) * NPAIR) // NST
                while ap < want:
                    emit_attnv_pair(r - 1, ap)
                    ap += 1
            while ap < NPAIR:
                emit_attnv_pair(r - 1, ap)
                ap += 1
            emit_tree(r)

        # ================= drain =================
        r = NCHUNK - 1
        ph_all[r] = [
            ps_acc.tile([128, 512], F32, tag="acc", name=f"h{r}_{hh}")
            for hh in range(2)
        ]
        emit_hcopy_proj(r - 1)
        for p in range(NPAIR):
            emit_attnv_pair(r, p)
        emit_z(r)
        emit_hcopy_proj(r)
        nc.sync.dma_start(out=zout[:, :], in_=z_sb)

    if not nc.is_finalized():
        nc.finalize()
    return nc


_NC_CACHE = None


def _get_nc():
    global _NC_CACHE
    if _NC_CACHE is None:
        _NC_CACHE = build_program()
    return _NC_CACHE


def kernel(x, norm_w, norm_b, w_qkv, w_proj, b_proj):
    global LAST_RESULT
    x = np.asarray(x, dtype=np.float32)
    norm_w = np.asarray(norm_w, dtype=np.float32)
    norm_b = np.asarray(norm_b, dtype=np.float32)
    w_qkv = np.asarray(w_qkv, dtype=np.float32)
    w_proj = np.asarray(w_proj, dtype=np.float32)
    b_proj = np.asarray(b_proj, dtype=np.float32)

    s1 = 1.0 / math.sqrt(math.sqrt(CH))
    bf16 = ml_dtypes.bfloat16
    f8 = ml_dtypes.float8_e4m3
    mgrp = (np.arange(128)[:, None] // 16 == np.arange(8)[None, :]).astype(bf16)
    mgrpT = np.ascontiguousarray(mgrp.T)

    # host-side GroupNorm stats (for the exact v-bias compensation)
    xr = x.reshape(B, G, C // G * N)
    mu_g = xr.mean(axis=2)
    var_g = xr.var(axis=2)
    rstd_g = 1.0 / np.sqrt(var_g + EPS)
    mu_c = np.repeat(mu_g, C // G, axis=1)      # [B, C]
    rstd_c = np.repeat(rstd_g, C // G, axis=1)  # [B, C]

    in_maps = []
    for core in range(NCORES):
        b, h = divmod(core, NH)
        rows = w_qkv[384 * h : 384 * (h + 1)]          # (384, 512)
        wfold = rows * norm_w[None, :]
        bias0 = rows @ norm_b
        scale_vec = np.concatenate(
            [np.full(128, s1 * ALPHA), np.full(128, s1 / ALPHA), np.ones(128)]
        ).astype(np.float32)
        wfold = wfold * scale_vec[:, None]
        bias0 = bias0 * scale_vec
        # wq2[pr, p, i, o] = wfold[o, 128*(2pr+i)+p]
        wq2 = np.ascontiguousarray(
            wfold.T.reshape(2, 2, 128, 384).transpose(0, 2, 1, 3).astype(bf16)
        )
        bqkv = np.ascontiguousarray(
            bias0[:256].reshape(2, 128).T.astype(np.float32)
        )
        wprojT = np.ascontiguousarray(
            w_proj[:, 128 * h : 128 * (h + 1)].T.astype(bf16)
        )
        xb = x[b].reshape(C, N)
        x2 = np.ascontiguousarray(
            xb.reshape(2, 2, 128, N).transpose(0, 2, 1, 3).astype(f8)
        )
        in_maps.append(
            {
                "x2": x2,
                "wq2": wq2,
                "bqkv": bqkv,
                "wprojT": wprojT,
                "mgrp": mgrp,
                "mgrpT": mgrpT,
            }
        )

    nc = _get_nc()
    res = run_bass_kernel_spmd(
        nc,
        in_maps,
        list(range(NCORES)),
        trace=TRACE,
        trace_cores=TRACE_CORES if TRACE else None,
    )
    LAST_RESULT = res

    out = np.empty((B, C, N), dtype=np.float32)
    for b in range(B):
        acc = x[b].reshape(C, N) + b_proj[:, None]
        for h in range(NH):
            r = res.results[4 * b + h]
            acc = acc + r["partial"].astype(np.float32) / r["zout"]
            # v-bias compensation: attention rows sum to 1
            rows_v = w_qkv[384 * h + 256 : 384 * (h + 1)]
            wv_fold = rows_v * norm_w[None, :]
            bias_v = rows_v @ norm_b - (wv_fold * rstd_c[b]) @ mu_c[b]
            acc = acc + (w_proj[:, 128 * h : 128 * (h + 1)] @ bias_v)[:, None]
        out[b] = acc
    return out.reshape(B, C, 64, 64)


# revision 13
# speedup vs baseline: 1.1606x; 1.1543x over previous
"""AttentionBlock (GroupNorm -> qkv conv1x1 -> 4-head attention -> proj + residual)
on 8 Trainium2 NeuronCores.

Sharding: B*NH = 2*4 = 8 (batch, head) pairs -> one per core.

Per core (head h of batch b):
  - x shipped as fp8e4 [512, 4096] in kt-pair layout (DMA halved vs bf16)
  - GroupNorm folded into runtime weight scaling: W'' = W'(gamma,s1-folded)
    * rstd[channel]; bias = -W''@mu (+ beta fold) applied during the qk
    PSUM->SBUF copies on DVE.  No normalization pass over x.
  - qkv GEMM in fp8 DoubleRow (contraction 256/pass).  q rows folded x4,
    k rows /4 so fp8 weights stay in the normal range (cancels in q.k).
  - scoresT[s,t] = k_tile.T @ q (bf16, s on partitions)
  - exp on ACT only: fp8e4 out with +2.8 offset (cancels in partial/Z),
    written into [128, 2, TCHUNK] pair tiles
  - attn@v: fp8 DoubleRow over s-tile pairs (2x MACs/cycle)
  - Z[t]: pairs 0..Z_PE-1 summed by DoubleRow ones-matmuls on PE; pairs
    Z_PE..15 by a DVE fp16 add tree -> esum; esum joined into the same
    PSUM group by an fp16 ones-matmul.
  - proj on PE (bf16); partial shipped bf16; v-bias compensated on host
    (attention rows sum to 1, so it shifts h/Z by exactly bias_v).
Host: out[b] = x + b_proj + sum_h (partial_h/Z_h + Wp_h @ bias_v_h).
"""

import math
from contextlib import ExitStack

import ml_dtypes
import numpy as np

import concourse.bacc as bacc
import concourse.bass as bass
import concourse.mybir as mybir
import concourse.tile as tile
from concourse.bass_utils import run_bass_kernel_spmd

C = 512
NH = 4
G = 32
EPS = 1e-5
N = 4096
CH = 128
B = 2
NCORES = 8
TCHUNK = 1024
NCHUNK = N // TCHUNK     # 4
NST = N // 128           # 32
NPAIR = NST // 2         # 16
Z_PE = 10                # s-tile pairs of Z summed on PE (rest on DVE tree)
EXPB = 2.8               # exp offset: e^(score+2.8) stays within fp8e4 range
ALPHA = 4.0              # q-row fold (k rows get 1/ALPHA); cancels in q.k

F32 = mybir.dt.float32
BF16 = mybir.dt.bfloat16
F16 = mybir.dt.float16
F8 = mybir.dt.float8e4
DR = mybir.MatmulPerfMode.DoubleRow
AF = mybir.ActivationFunctionType

TRACE = False
TRACE_CORES = [0]
LAST_RESULT = None


def build_program():
    nc = bacc.Bacc()

    # x2[pr, p, i, n] = x[128*(2pr+i)+p, n]  (kt-pair layout for DoubleRow)
    x2d = nc.declare_dram_parameter("x2", [2, 128, 2, N], F8, isOutput=False)
    # wq2[pr, p, i, o] = W'[o, 128*(2pr+i)+p] (gamma, s1, alpha folded)
    wq2d = nc.declare_dram_parameter("wq2", [2, 128, 2, 384], BF16, isOutput=False)
    bqkv = nc.declare_dram_parameter("bqkv", [128, 2], F32, isOutput=False)
    wprojT = nc.declare_dram_parameter("wprojT", [CH, C], BF16, isOutput=False)
    mgrp = nc.declare_dram_parameter("mgrp", [128, 8], BF16, isOutput=False)
    mgrpT = nc.declare_dram_parameter("mgrpT", [8, 128], BF16, isOutput=False)
    ident = nc.declare_dram_parameter("ident", [128, 128], BF16, isOutput=False)
    partial = nc.declare_dram_parameter("partial", [C, N], BF16, isOutput=True)
    zout = nc.declare_dram_parameter("zout", [1, N], F32, isOutput=True)

    with tile.TileContext(nc) as tc, ExitStack() as ctx:
        consts = ctx.enter_context(tc.tile_pool(name="consts", bufs=1))
        gn = ctx.enter_context(tc.tile_pool(name="gn", bufs=1))
        xpool = ctx.enter_context(tc.tile_pool(name="xpool", bufs=2))
        spool = ctx.enter_context(tc.tile_pool(name="spool", bufs=8))
        qkp = ctx.enter_context(tc.tile_pool(name="qkp", bufs=1))
        epool = ctx.enter_context(tc.tile_pool(name="epool", bufs=32))
        trpool = ctx.enter_context(tc.tile_pool(name="trpool", bufs=14))
        hpool = ctx.enter_context(tc.tile_pool(name="hpool", bufs=2))
        opool = ctx.enter_context(tc.tile_pool(name="opool", bufs=3))
        zsp = ctx.enter_context(tc.tile_pool(name="zsp", bufs=1))
        ps_sc = ctx.enter_context(tc.tile_pool(name="ps_sc", bufs=2, space="PSUM"))
        ps_acc = ctx.enter_context(tc.tile_pool(name="ps_acc", bufs=2, space="PSUM"))
        ps_mm2 = ctx.enter_context(tc.tile_pool(name="ps_mm2", bufs=2, space="PSUM"))

        # ---- constants / weights (gpsimd queue so sync handles x) ----
        mgrp_sb = consts.tile([128, 8], BF16, tag="mgrp")
        nc.gpsimd.dma_start(out=mgrp_sb, in_=mgrp[:, :])
        mgrpT_sb = consts.tile([8, 128], BF16, tag="mgrpT")
        nc.gpsimd.dma_start(out=mgrpT_sb, in_=mgrpT[:, :])
        wbf = []
        for pr in range(2):
            wt = consts.tile([128, 2, 384], BF16, tag=f"wb{pr}")
            nc.gpsimd.dma_start(out=wt, in_=wq2d[pr])
            wbf.append(wt)
        bq_sb = consts.tile([128, 2], F32, tag="bq")
        nc.gpsimd.dma_start(out=bq_sb, in_=bqkv[:, :])
        wp_sb = consts.tile([CH, C], BF16, tag="wp")
        nc.gpsimd.dma_start(out=wp_sb, in_=wprojT[:, :])
        ident_sb = consts.tile([128, 128], BF16, tag="ident")
        nc.gpsimd.dma_start(out=ident_sb, in_=ident[:, :])
        ones2 = consts.tile([128, 2, 16], F8, tag="ones2")
        nc.vector.memset(ones2, 1.0)
        ones1 = consts.tile([128, 1], F16, tag="ones1")
        nc.vector.memset(ones1, 1.0)
        bexp = consts.tile([128, 1], F32, tag="bexp")
        nc.vector.memset(bexp, EXPB)
        beps = consts.tile([8, 1], F32, tag="beps")
        nc.vector.memset(beps, EPS)

        # ---- x loads (8 half-DMAs on both HWDGE queues) + pipelined stats:
        # tiles 0,1 (pr=0) -> DVE bn_stats; tiles 2,3 (pr=1) -> ACT accum ----
        xt = [xpool.tile([128, 2, N], F8, tag="xt", name=f"x{pr}")
              for pr in range(2)]
        H2 = N // 2
        for hv in range(2):
            for kt in (2, 3, 0, 1):       # ACT tiles first (longer path)
                pr, i = divmod(kt, 2)
                eng = nc.scalar if pr == 1 else nc.sync
                eng.dma_start(
                    out=xt[pr][:, i, hv * H2 : (hv + 1) * H2],
                    in_=x2d[pr, :, i, hv * H2 : (hv + 1) * H2],
                )

        stats_all = gn.tile([128, 8], F32, tag="stats_all")
        sq_scr = gn.tile([128, H2], F8, tag="sq_scr")
        sxp = {}
        st_t = {}
        for hv in range(2):
            for kt in range(4):
                pr, i = divmod(kt, 2)
                xsl = xt[pr][:, i, hv * H2 : (hv + 1) * H2]
                if kt < 2:
                    if hv == 0:
                        st_t[kt] = spool.tile([128, 8, 6], F32, tag="bst",
                                              name=f"bst{kt}")
                    xv = xsl.rearrange("p (s f) -> p s f", f=512)
                    for s in range(4):
                        nc.vector.bn_stats(out=st_t[kt][:, 4 * hv + s, :],
                                           in_=xv[:, s, :])
                else:
                    s2 = spool.tile([128, 1], F32, tag="sx", name=f"s2_{kt}_{hv}")
                    nc.scalar.activation(out=sq_scr, in_=xsl, func=AF.Square,
                                         accum_out=s2)
                    s1t = spool.tile([128, 1], F32, tag="sx", name=f"s1_{kt}_{hv}")
                    nc.scalar.activation(out=sq_scr, in_=xsl, func=AF.Identity,
                                         accum_out=s1t)
                    sxp[(kt, hv)] = (s1t, s2)
        for kt in range(4):
            if kt < 2:
                mv = spool.tile([128, 2], F32, tag="mv", name=f"mv{kt}")
                nc.vector.bn_aggr(out=mv, in_=st_t[kt])
                nc.vector.tensor_copy(out=stats_all[:, kt : kt + 1], in_=mv[:, 0:1])
                nc.vector.tensor_mul(
                    out=stats_all[:, 4 + kt : 5 + kt], in0=mv[:, 0:1], in1=mv[:, 0:1]
                )
                nc.vector.tensor_add(
                    out=stats_all[:, 4 + kt : 5 + kt],
                    in0=stats_all[:, 4 + kt : 5 + kt],
                    in1=mv[:, 1:2],
                )
            else:
                (s1a, s2a), (s1b, s2b) = sxp[(kt, 0)], sxp[(kt, 1)]
                nc.vector.tensor_add(out=s1a, in0=s1a, in1=s1b)
                nc.vector.tensor_add(out=s2a, in0=s2a, in1=s2b)
                nc.vector.tensor_scalar_mul(
                    out=stats_all[:, kt : kt + 1], in0=s1a, scalar1=1.0 / N
                )
                nc.vector.tensor_scalar_mul(
                    out=stats_all[:, 4 + kt : 5 + kt], in0=s2a, scalar1=1.0 / N
                )

        # ---- group aggregation: mean_g, rstd_g -> per-channel sc_all ----
        stats16 = gn.tile([128, 8], BF16, tag="stats16")
        nc.vector.tensor_copy(out=stats16, in_=stats_all)
        ps_t = ps_mm2.tile([8, 8], F32, tag="mm2", name="gnps")
        nc.tensor.matmul(ps_t, lhsT=mgrp_sb, rhs=stats16, start=True, stop=True)
        gs = gn.tile([8, 8], F32, tag="gs8")
        nc.vector.tensor_scalar_mul(out=gs, in0=ps_t, scalar1=1.0 / 16.0)
        gvals = gn.tile([8, 8], F32, tag="gvals")
        nc.vector.tensor_copy(out=gvals[:, 0:4], in_=gs[:, 0:4])
        varg = gn.tile([8, 4], F32, tag="varg")
        nc.vector.tensor_mul(out=varg, in0=gs[:, 0:4], in1=gs[:, 0:4])
        nc.vector.tensor_sub(out=varg, in0=gs[:, 4:8], in1=varg)
        # rstd = exp(-0.5*ln(var+eps)) : stays in the exp/ln ACT table set
        nc.scalar.activation(out=varg, in_=varg, func=AF.Ln, bias=beps)
        nc.scalar.activation(
            out=gvals[:, 4:8], in_=varg, func=AF.Exp, scale=-0.5
        )
        gvals16 = gn.tile([8, 8], BF16, tag="gvals16")
        nc.vector.tensor_copy(out=gvals16, in_=gvals)
        ps_t2 = ps_mm2.tile([128, 8], F32, tag="mm2", name="gnps2")
        nc.tensor.matmul(ps_t2, lhsT=mgrpT_sb, rhs=gvals16, start=True, stop=True)
        sc_all = gn.tile([128, 8], F32, tag="scall")
        nc.vector.tensor_copy(out=sc_all, in_=ps_t2)

        # ---- W'' = W' * rstd (fp8), msc = mean*rstd, qk bias GEMV ----
        w8 = []
        for pr in range(2):
            w8t = qkp.tile([128, 2, 384], F8, tag=f"w8{pr}")
            for i in range(2):
                kt = 2 * pr + i
                nc.vector.tensor_scalar_mul(
                    out=w8t[:, i, :],
                    in0=wbf[pr][:, i, :],
                    scalar1=sc_all[:, 4 + kt : 5 + kt],
                )
            w8.append(w8t)
        msc16 = gn.tile([128, 4], BF16, tag="msc16")
        nc.vector.tensor_mul(out=msc16, in0=sc_all[:, 0:4], in1=sc_all[:, 4:8])
        bias_qk = []
        for j in range(2):
            ps_b = ps_mm2.tile([128, 1], F32, tag="mm2", name=f"gemv{j}")
            for kt in range(4):
                pr, i = divmod(kt, 2)
                nc.tensor.matmul(
                    ps_b,
                    lhsT=wbf[pr][:, i, 128 * j : 128 * (j + 1)],
                    rhs=msc16[:, kt : kt + 1],
                    start=(kt == 0),
                    stop=(kt == 3),
                )
            bj = gn.tile([128, 1], F32, tag=f"bias{j}")
            nc.vector.tensor_sub(out=bj, in0=bq_sb[:, j : j + 1], in1=ps_b)
            bias_qk.append(bj)

        # ---- qkv chunk GEMM helper (DoubleRow) ----
        q_sb = qkp.tile([128, N], BF16, tag="q_sb")
        k_sb = qkp.tile([128, N], BF16, tag="k_sb")
        v_sb = qkp.tile([128, N], BF16, tag="v_sb")
        vT2 = qkp.tile([128, NST, 128], F8, tag="vT2")

        def qkv_chunk(j, ch):
            ps = ps_mm2.tile([128, 512], F32, tag="mm2", name=f"qkv{j}_{ch}")
            for pr in range(2):
                nc.tensor.matmul(
                    ps,
                    lhsT=w8[pr][:, :, 128 * j : 128 * (j + 1)],
                    rhs=xt[pr][:, :, 512 * ch : 512 * (ch + 1)],
                    start=(pr == 0),
                    stop=(pr == 1),
                    perf_mode=DR,
                )
            dst = (q_sb, k_sb, v_sb)[j]
            if j < 2:
                nc.vector.tensor_scalar(
                    out=dst[:, 512 * ch : 512 * (ch + 1)],
                    in0=ps,
                    scalar1=bias_qk[j],
                    scalar2=None,
                    op0=mybir.AluOpType.add,
                )
            else:
                nc.vector.tensor_copy(
                    out=dst[:, 512 * ch : 512 * (ch + 1)], in_=ps
                )
                # transpose the 4 s-tiles of this chunk on the PE, then pack
                # bf16 psum -> fp8 vT2 in one DVE copy
                ps_tr = ps_acc.tile([128, 4, 128], BF16, tag="acc",
                                    name=f"tr{ch}")
                for l in range(4):
                    stt = 4 * ch + l
                    nc.tensor.transpose(
                        ps_tr[:, l, :],
                        v_sb[:, 128 * stt : 128 * (stt + 1)],
                        ident_sb,
                    )
                nc.vector.tensor_copy(
                    out=vT2[:, 4 * ch : 4 * (ch + 1), :], in_=ps_tr
                )

        # ---- round machinery ----
        et_all = []          # per round: list of NPAIR pair tiles
        esums = [None] * NCHUNK
        ph_all = [None] * NCHUNK

        def emit_scores(r, stt):
            ps = ps_sc.tile([128, TCHUNK], F32, tag="sc")
            ksl = k_sb[:, 128 * stt : 128 * (stt + 1)]
            t0 = r * TCHUNK
            for hh in range(2):
                nc.tensor.matmul(
                    ps[:, 512 * hh : 512 * (hh + 1)],
                    lhsT=ksl,
                    rhs=q_sb[:, t0 + 512 * hh : t0 + 512 * (hh + 1)],
                    start=True,
                    stop=True,
                )
            if stt % 2 == 0:
                et = epool.tile([128, 2, TCHUNK], F8, tag="et")
                et_all[r].append(et)
            nc.scalar.activation(
                out=et_all[r][stt // 2][:, stt % 2, :],
                in_=ps,
                func=AF.Exp,
                bias=bexp,
            )

        def emit_attnv_pair(r, p):
            ph = ph_all[r]
            for hh in range(2):
                nc.tensor.matmul(
                    ph[hh],
                    lhsT=vT2[:, 2 * p : 2 * p + 2, :],
                    rhs=et_all[r][p][:, :, 512 * hh : 512 * (hh + 1)],
                    start=(p == 0),
                    stop=(p == NPAIR - 1),
                    perf_mode=DR,
                )

        def emit_z(r):
            # both t-halves; PE pairs 0..Z_PE-1 + fp16 esum join
            for hh in range(2):
                zps = ps_mm2.tile([1, 512], F32, tag="mm2", name=f"z{r}_{hh}")
                for p in range(Z_PE):
                    nc.tensor.matmul(
                        zps,
                        lhsT=ones2[:, :, 0:1],
                        rhs=et_all[r][p][:, :, 512 * hh : 512 * (hh + 1)],
                        start=(p == 0),
                        stop=False,
                        perf_mode=DR,
                    )
                nc.tensor.matmul(
                    zps,
                    lhsT=ones1,
                    rhs=esums[r][:, 512 * hh : 512 * (hh + 1)],
                    start=False,
                    stop=True,
                )
                nc.vector.tensor_copy(
                    out=z_sb[:, r * TCHUNK + 512 * hh : r * TCHUNK + 512 * (hh + 1)],
                    in_=zps,
                )

        def emit_tree(r):
            # DVE fp16 tree over pairs Z_PE..15 of round r -> esums[r]
            tt = []
            for p in range(Z_PE, NPAIR):
                t_ = trpool.tile([128, TCHUNK], F16, tag="tr", name=f"t{r}_{p}")
                nc.vector.tensor_add(
                    out=t_, in0=et_all[r][p][:, 0, :], in1=et_all[r][p][:, 1, :]
                )
                tt.append(t_)
            while len(tt) > 1:
                nxt = []
                for a in range(0, len(tt) - 1, 2):
                    nc.vector.tensor_add(out=tt[a], in0=tt[a], in1=tt[a + 1])
                    nxt.append(tt[a])
                if len(tt) % 2 == 1:
                    nxt.append(tt[-1])
                tt = nxt
            esums[r] = tt[0]

        def emit_hcopy_proj(r):
            hsb = hpool.tile([128, TCHUNK], BF16, tag="h")
            for hh in range(2):
                nc.vector.tensor_copy(
                    out=hsb[:, 512 * hh : 512 * (hh + 1)], in_=ph_all[r][hh]
                )
            for ot in range(4):
                for hh in range(2):
                    psp = ps_mm2.tile([128, 512], F32, tag="mm2",
                                      name=f"pj{r}_{ot}_{hh}")
                    nc.tensor.matmul(
                        psp,
                        lhsT=wp_sb[:, 128 * ot : 128 * (ot + 1)],
                        rhs=hsb[:, 512 * hh : 512 * (hh + 1)],
                        start=True,
                        stop=True,
                    )
                    ob = opool.tile([128, 512], BF16, tag="ob")
                    nc.vector.tensor_copy(out=ob, in_=psp)
                    nc.sync.dma_start(
                        out=partial[
                            128 * ot : 128 * (ot + 1),
                            r * TCHUNK + 512 * hh : r * TCHUNK + 512 * (hh + 1),
                        ],
                        in_=ob,
                    )

        z_sb = zsp.tile([1, N], F32, tag="z_sb")

        # ================= round 0 =================
        et_all.append([])
        # k chunk 0, q chunks 0,1 first so scores can start
        qkv_chunk(1, 0)
        qkv_chunk(0, 0)
        qkv_chunk(0, 1)
        emit_scores(0, 0)
        emit_scores(0, 1)
        # stream remaining k chunks just ahead of their score tiles;
        # weave in v chunks (+ transposes) and q chunks 2,3
        fill0 = [("k", ch) for ch in range(1, 8)]
        extra0 = [("v", ch) for ch in range(8)] + [("q", 2), ("q", 3)]
        ei = 0
        for stt in range(2, NST):
            ch = stt // 4 + 1
            if fill0 and stt % 4 == 2 and ch < 8:
                qkv_chunk(1, ch)
                fill0.pop(0)
            if stt % 2 == 1 and ei < len(extra0):
                kind, ch2 = extra0[ei]
                qkv_chunk(2 if kind == "v" else 0, ch2)
                ei += 1
            emit_scores(0, stt)
        while ei < len(extra0):
            kind, ch2 = extra0[ei]
            qkv_chunk(2 if kind == "v" else 0, ch2)
            ei += 1
        emit_tree(0)

        # ================= rounds 1..3 =================
        ap3 = 0
        for r in range(1, NCHUNK):
            last = r == NCHUNK - 1
            et_all.append([])
            ph_all[r - 1] = [
                ps_acc.tile([128, 512], F32, tag="acc", name=f"h{r-1}_{hh}")
                for hh in range(2)
            ]
            if last:
                ph_all[r] = [
                    ps_acc.tile([128, 512], F32, tag="acc", name=f"h{r}_{hh}")
                    for hh in range(2)
                ]
            if r >= 2:
                emit_hcopy_proj(r - 2)
            # q chunks for rounds 2,3 streamed early in round 1
            pend_q = [("q", 4), ("q", 5), ("q", 6), ("q", 7)] if r == 1 else []
            ap = 0  # attnv pairs of round r-1 emitted
            # in the last round, compress attnv(r-1) into the first 20 slots
            # so h(r-1) copies + proj(r-1) can run in-round; chase attnv(r)
            # right behind its exps to shrink the drain
            denom = 20 if last else NST
            for stt in range(NST):
                emit_scores(r, stt)
                if pend_q and stt % 2 == 1:
                    qkv_chunk(0, pend_q.pop(0)[1])
                if stt == 10:
                    emit_z(r - 1)
                want = min(NPAIR, ((stt + 1) * NPAIR) // denom)
                while ap < want:
                    emit_attnv_pair(r - 1, ap)
                    ap += 1
                if last:
                    if stt == 21:
                        emit_hcopy_proj(r - 1)
                    while ap3 < min(max(0, (stt - 3) // 2), NPAIR - 1):
                        emit_attnv_pair(r, ap3)
                        ap3 += 1
            while ap < NPAIR:
                emit_attnv_pair(r - 1, ap)
                ap += 1
            emit_tree(r)

        # ================= drain =================
        r = NCHUNK - 1
        while ap3 < NPAIR:
            emit_attnv_pair(r, ap3)
            ap3 += 1
        emit_z(r)
        emit_hcopy_proj(r)
        nc.sync.dma_start(out=zout[:, :], in_=z_sb)

    if not nc.is_finalized():
        nc.finalize()
    return nc


_NC_CACHE = None


def _get_nc():
    global _NC_CACHE
    if _NC_CACHE is None:
        _NC_CACHE = build_program()
    return _NC_CACHE


def kernel(x, norm_w, norm_b, w_qkv, w_proj, b_proj):
    global LAST_RESULT
    x = np.asarray(x, dtype=np.float32)
    norm_w = np.asarray(norm_w, dtype=np.float32)
    norm_b = np.asarray(norm_b, dtype=np.float32)
    w_qkv = np.asarray(w_qkv, dtype=np.float32)
    w_proj = np.asarray(w_proj, dtype=np.float32)
    b_proj = np.asarray(b_proj, dtype=np.float32)

    s1 = 1.0 / math.sqrt(math.sqrt(CH))
    bf16 = ml_dtypes.bfloat16
    f8 = ml_dtypes.float8_e4m3
    mgrp = (np.arange(128)[:, None] // 16 == np.arange(8)[None, :]).astype(bf16)
    mgrpT = np.ascontiguousarray(mgrp.T)

    # host-side GroupNorm stats (for the exact v-bias compensation)
    xr = x.reshape(B, G, C // G * N)
    mu_g = xr.mean(axis=2)
    var_g = xr.var(axis=2)
    rstd_g = 1.0 / np.sqrt(var_g + EPS)
    mu_c = np.repeat(mu_g, C // G, axis=1)      # [B, C]
    rstd_c = np.repeat(rstd_g, C // G, axis=1)  # [B, C]

    in_maps = []
    for core in range(NCORES):
        b, h = divmod(core, NH)
        rows = w_qkv[384 * h : 384 * (h + 1)]          # (384, 512)
        wfold = rows * norm_w[None, :]
        bias0 = rows @ norm_b
        scale_vec = np.concatenate(
            [np.full(128, s1 * ALPHA), np.full(128, s1 / ALPHA), np.ones(128)]
        ).astype(np.float32)
        wfold = wfold * scale_vec[:, None]
        bias0 = bias0 * scale_vec
        # wq2[pr, p, i, o] = wfold[o, 128*(2pr+i)+p]
        wq2 = np.ascontiguousarray(
            wfold.T.reshape(2, 2, 128, 384).transpose(0, 2, 1, 3).astype(bf16)
        )
        bqkv = np.ascontiguousarray(
            bias0[:256].reshape(2, 128).T.astype(np.float32)
        )
        wprojT = np.ascontiguousarray(
            w_proj[:, 128 * h : 128 * (h + 1)].T.astype(bf16)
        )
        xb = x[b].reshape(C, N)
        x2 = np.ascontiguousarray(
            xb.reshape(2, 2, 128, N).transpose(0, 2, 1, 3).astype(f8)
        )
        in_maps.append(
            {
                "x2": x2,
                "wq2": wq2,
                "bqkv": bqkv,
                "wprojT": wprojT,
                "mgrp": mgrp,
                "mgrpT": mgrpT,
                "ident": np.eye(128, dtype=bf16),
            }
        )

    nc = _get_nc()
    res = run_bass_kernel_spmd(
        nc,
        in_maps,
        list(range(NCORES)),
        trace=TRACE,
        trace_cores=TRACE_CORES if TRACE else None,
    )
    LAST_RESULT = res

    out = np.empty((B, C, N), dtype=np.float32)
    for b in range(B):
        acc = x[b].reshape(C, N) + b_proj[:, None]
        for h in range(NH):
            r = res.results[4 * b + h]
            acc = acc + r["partial"].astype(np.float32) / r["zout"]
            # v-bias compensation: attention rows sum to 1
            rows_v = w_qkv[384 * h + 256 : 384 * (h + 1)]
            wv_fold = rows_v * norm_w[None, :]
            bias_v = rows_v @ norm_b - (wv_fold * rstd_c[b]) @ mu_c[b]
            acc = acc + (w_proj[:, 128 * h : 128 * (h + 1)] @ bias_v)[:, None]
        out[b] = acc
    return out.reshape(B, C, 64, 64)
